# revision 7
# baseline (speedup 1.0000x reference)
"""Port-Hamiltonian model forward pass (dstate/dt) as a Bass/Tile kernel on
8 TRN2 NeuronCores, pure data-parallel over the batch.

v2: 3 scalar passes/slice (was 4) + fp8 DoubleRow matmuls (was bf16).

Math per sample (feature-major on chip, batch in the matmul moving dim):
    z1   = W1.T s + b1                        [512]   bf16 hi/lo matmul
    s1m  = sigmoid(-z1) = 1 - sigma(z1)       f16     (Sigmoid table, scale=-1)
    m1   = ln(s1m) = -softplus(z1)            fp8     (Ln table)
    z2ps = (-s_w W2).T m1 + s_w b2 = s_w z2   f32     fp8 DoubleRow + K=2 const mm
    t2   = tanh(z2/2)                         fp8     (Tanh table, scale=1/(2 s_w))
    ups  = (s_u/2 Wb).T t2 + s_u u0 = s_u u   f32     fp8 DoubleRow + K=2 const mm
           (Wb[j,i] = W2[i,j] w3[j],  u0 = colsum(Wb)/2,  sigma(z2) = .5 + .5 t2)
    g1   = (s1m - 1) * ups = -s_u sigma(z1) u bf16    one scalar_tensor_tensor
    out  = (-(M@W1)/s_u).T g1 + GM.T a        f32     bf16 matmuls
  where M = [[0, 1], [-1, -damping]], GM carries Gw/Gb on [a_hi; a_lo; 1].

Schedule: two activation-table sets alternate in supertile phases so the
scalar engine (the bottleneck: 3 x 2048 elem/slice at 1 elem/lane/cycle)
never waits: S-phase (sigmoid_and_others) runs sigma(-z1) for supertile k
and tanh + backward for supertile k-1; L-phase (natural_log_exp) runs ln +
the z2 matmuls, parking z2 in SBUF f16 across the table switch.
"""

import numpy as np
import ml_dtypes

B = 131072
S = 2
H = 512
E = 8
NCORES = 8
BC = B // NCORES   # 16384 samples per core
NSLICE = 512       # batch slice (matmul moving free dim / PSUM bank)
NS = BC // NSLICE  # 32 slices
NSUP = 2           # supertiles per core (table-set phases)
NSS = NS // NSUP   # slices per supertile
HC = H // 128      # 4 hidden-dim chunks of 128 partitions
LG = 2             # slices per x/a DMA load group

S_W = 32.0         # z2 fp8 weight scale
S_U = 512.0        # u fp8 weight scale

BF16 = ml_dtypes.bfloat16
F8 = ml_dtypes.float8_e4m3
F16 = np.float16

_cached = {}
last_results = None  # test.py introspects this for profiling info


def _pin_act_tables():
    """Restrict the activation-table chooser to the two sets this kernel
    wants so insert_act_table_loads doesn't ping-pong per slice."""
    import functools
    import concourse.hw_specs as hw_specs
    import concourse.bacc as bacc

    if getattr(hw_specs.get_activation_tables, "_ph_pinned", False):
        return
    orig = hw_specs.get_activation_tables
    KEEP = {"natural_log_exp_and_others", "sigmoid_and_others"}

    @functools.cache
    def pinned(module_arch):
        full = orig(module_arch)
        return {n: (f if n in KEEP else set()) for n, f in full.items()}

    pinned._ph_pinned = True
    hw_specs.get_activation_tables = pinned
    bacc.get_activation_tables = pinned


def _build_nc():
    import concourse.bacc as bacc
    import concourse.mybir as mybir
    import concourse.tile as tile

    _pin_act_tables()

    f32 = mybir.dt.float32
    bf16 = mybir.dt.bfloat16
    f16 = mybir.dt.float16
    fp8 = mybir.dt.float8e4
    SUB = mybir.AluOpType.subtract
    MUL = mybir.AluOpType.mult
    SIG = mybir.ActivationFunctionType.Sigmoid
    TANH = mybir.ActivationFunctionType.Tanh
    LN = mybir.ActivationFunctionType.Ln
    DR = mybir.MatmulPerfMode.DoubleRow

    nc = bacc.Bacc("TRN2", target_bir_lowering=False, debug=False)

    xT_d = nc.dram_tensor("xT", [7, BC], bf16, kind="ExternalInput")
    aT_d = nc.dram_tensor("aT", [17, BC], bf16, kind="ExternalInput")
    # W1-aug row-tiled: rows 32j+r (r<7) hold [W1hi;W1hi;W1lo;b1][r, 128j:128j+128]
    w1rt_d = nc.dram_tensor("w1rt", [128, 128], bf16, kind="ExternalInput")
    w2q_d = nc.dram_tensor("w2q", [128, HC, H], fp8, kind="ExternalInput")
    wuq_d = nc.dram_tensor("wuq", [128, HC, H], fp8, kind="ExternalInput")
    w1f_d = nc.dram_tensor("w1f", [128, HC, S], bf16, kind="ExternalInput")
    gm_d = nc.dram_tensor("gm", [17, S], bf16, kind="ExternalInput")
    # const-fold weights, row-tiled: rows 32c+{0,1} = hi/lo pair for chunk c
    cwb_d = nc.dram_tensor("cwb", [128, 128], bf16, kind="ExternalInput")
    cwu_d = nc.dram_tensor("cwu", [128, 128], bf16, kind="ExternalInput")
    outT_d = nc.dram_tensor("outT", [S, BC], f32, kind="ExternalOutput")

    with tile.TileContext(nc) as tc:
        with (
            tc.tile_pool(name="consts", bufs=1) as consts,
            tc.tile_pool(name="work", bufs=2) as work,
            tc.tile_pool(name="ps", bufs=1, space="PSUM") as ps,
        ):
            # ---- constants ----
            w1rt = consts.tile([128, 128], bf16)
            nc.sync.dma_start(w1rt[:], w1rt_d[:])
            w2q = consts.tile([128, HC, H], fp8)
            nc.sync.dma_start(w2q[:], w2q_d[:])
            wuq = consts.tile([128, HC, H], fp8)
            nc.sync.dma_start(wuq[:], wuq_d[:])
            w1f = consts.tile([128, HC, S], bf16)
            nc.sync.dma_start(w1f[:], w1f_d[:])
            gm = consts.tile([17, S], bf16)
            nc.sync.dma_start(gm[:], gm_d[:])
            cwb = consts.tile([128, 128], bf16)
            nc.sync.dma_start(cwb[:], cwb_d[:])
            cwu = consts.tile([128, 128], bf16)
            nc.sync.dma_start(cwu[:], cwu_d[:])
            ones2 = consts.tile([128, NSLICE], bf16)
            nc.vector.memset(ones2[:], 1.0)
            # ln-guard bias: ln(s1m + 1e-12) stays finite even if s1m
            # underflows to 0 (would need z1 > ~17; belt and suspenders)
            epsb = consts.tile([128, 1], f32)
            nc.vector.memset(epsb[:], 1e-12)

            def load_x_rt(g, tag):
                """x load group replicated at partition offsets 0/32/64/96
                for row-tiled z1 matmuls."""
                csl = slice(g * LG * NSLICE, (g + 1) * LG * NSLICE)
                x_t = work.tile(
                    [128, LG * NSLICE], bf16, tag="xa", bufs=2, name=f"x{tag}"
                )
                for j in range(4):
                    nc.sync.dma_start(x_t[32 * j : 32 * j + 7, :], xT_d[:, csl])
                return x_t

            # HAM warmup: dummy matmuls fill the startup gap while weight
            # DMAs stream in, pushing the PE clock gate toward 8/8.
            warm = work.tile([128, NSLICE], bf16, tag="warm", bufs=1)
            nc.vector.memset(warm[:], 0.0)
            wp = ps.tile([128, NSLICE], f32, tag="psb", bufs=4, name="warmps")
            for i in range(20):
                nc.tensor.matmul(
                    wp[:], warm[:, :128], warm[:], start=True, stop=True,
                    skip_group_check=True,
                )

            # rings that persist across a phase boundary
            s1m_t = [None] * NS   # sigmoid(-z1), f16
            z2s_t = [None] * NS   # s_w * z2, f16
            x_cur = [None]
            a_cur = [None]

            for k in range(NSUP + 1):
                # ======== S-phase (sigmoid_and_others: Sigmoid + Tanh) ======
                for p in range(NSS):
                    if k < NSUP:
                        s = k * NSS + p
                        if s % LG == 0:
                            x_cur[0] = load_x_rt(s // LG, f"s{s}")
                        x_t = x_cur[0]
                        z1p = ps.tile(
                            [128, HC * NSLICE], f32, tag="psa", bufs=1,
                            name=f"z1p{s}",
                        )
                        for j in range(4):
                            nc.tensor.matmul(
                                z1p[:, j * NSLICE : (j + 1) * NSLICE],
                                w1rt[32 * j : 32 * j + 7, :],
                                x_t[
                                    32 * j : 32 * j + 7,
                                    (s % LG) * NSLICE : (s % LG + 1) * NSLICE,
                                ],
                                start=True,
                                stop=True,
                                tile_position=(32 * j, 0),
                            )
                        s1m_t[s] = work.tile(
                            [128, HC, NSLICE], f16, tag="sg", bufs=NSS + 3,
                            name=f"s1m{s}",
                        )
                        nc.scalar.activation(s1m_t[s][:], z1p[:], SIG, scale=-1.0)

                    if k > 0:
                        s2 = (k - 1) * NSS + p
                        if s2 % LG == 0:
                            a_t = work.tile(
                                [17, LG * NSLICE], bf16, tag="aa", bufs=2,
                                name=f"aa{s2}",
                            )
                            nc.sync.dma_start(
                                a_t[:],
                                aT_d[:, s2 * NSLICE : (s2 + LG) * NSLICE],
                            )
                            a_cur[0] = a_t
                        a_t = a_cur[0]

                        t2 = work.tile(
                            [128, HC, NSLICE], fp8, tag="t2", bufs=2,
                            name=f"t2_{s2}",
                        )
                        nc.scalar.activation(
                            t2[:], z2s_t[s2][:], TANH, scale=1.0 / (2.0 * S_W)
                        )
                        g1 = work.tile(
                            [128, HC, NSLICE], bf16, tag="g1", bufs=2,
                            name=f"g1_{s2}",
                        )
                        ups = []
                        for c in range(HC):
                            up = ps.tile(
                                [128, NSLICE], f32, tag="psb", bufs=4,
                                name=f"up{s2}_{c}",
                            )
                            for kp in range(2):
                                nc.tensor.matmul(
                                    up[:],
                                    wuq[:, 2 * kp : 2 * kp + 2,
                                        c * 128 : (c + 1) * 128],
                                    t2[:, 2 * kp : 2 * kp + 2, :],
                                    start=(kp == 0),
                                    stop=False,
                                    perf_mode=DR,
                                    skip_group_check=True,
                                )
                            ups.append(up)
                        for c in range(HC):  # u0 fold: 4 concurrent K=2 mms
                            nc.tensor.matmul(
                                ups[c][:],
                                cwu[32 * c : 32 * c + 2, :],
                                ones2[32 * c : 32 * c + 2, :],
                                start=False,
                                stop=True,
                                tile_position=(32 * c, 0),
                                skip_group_check=True,
                            )
                        for c in range(HC):
                            nc.vector.scalar_tensor_tensor(
                                g1[:, c, :],
                                s1m_t[s2][:, c, :],
                                1.0,
                                ups[c][:],
                                SUB,
                                MUL,
                            )
                        s1m_t[s2] = None

                        op = ps.tile(
                            [S, NSLICE], f32, tag="psb", bufs=4, name=f"op{s2}"
                        )
                        for c in range(HC):
                            nc.tensor.matmul(
                                op[:],
                                w1f[:, c, :],
                                g1[:, c, :],
                                start=(c == 0),
                                stop=False,
                                skip_group_check=True,
                            )
                        nc.tensor.matmul(
                            op[:],
                            gm[:],
                            a_t[:, (s2 % LG) * NSLICE : (s2 % LG + 1) * NSLICE],
                            start=False,
                            stop=True,
                            skip_group_check=True,
                        )
                        o_t = work.tile(
                            [S, NSLICE], f32, tag="osb", bufs=2, name=f"ot{s2}"
                        )
                        nc.vector.tensor_copy(o_t[:], op[:])
                        nc.sync.dma_start(
                            outT_d[:, s2 * NSLICE : (s2 + 1) * NSLICE], o_t[:]
                        )

                # ======== L-phase (natural_log_exp_and_others: Ln) ==========
                if k < NSUP:
                    for p in range(NSS):
                        s = k * NSS + p
                        m1 = work.tile(
                            [128, HC, NSLICE], fp8, tag="m1", bufs=2,
                            name=f"m1_{s}",
                        )
                        nc.scalar.activation(
                            m1[:], s1m_t[s][:], LN, bias=epsb[:]
                        )
                        z2s_t[s] = work.tile(
                            [128, HC, NSLICE], f16, tag="z2s", bufs=NSS + 2,
                            name=f"z2s{s}",
                        )
                        z2ps = []
                        for c in range(HC):
                            z2p = ps.tile(
                                [128, NSLICE], f32, tag="psb", bufs=4,
                                name=f"z2p{s}_{c}",
                            )
                            for kp in range(2):
                                nc.tensor.matmul(
                                    z2p[:],
                                    w2q[:, 2 * kp : 2 * kp + 2,
                                        c * 128 : (c + 1) * 128],
                                    m1[:, 2 * kp : 2 * kp + 2, :],
                                    start=(kp == 0),
                                    stop=False,
                                    perf_mode=DR,
                                    skip_group_check=True,
                                )
                            z2ps.append(z2p)
                        for c in range(HC):  # b2 fold: 4 concurrent K=2 mms
                            nc.tensor.matmul(
                                z2ps[c][:],
                                cwb[32 * c : 32 * c + 2, :],
                                ones2[32 * c : 32 * c + 2, :],
                                start=False,
                                stop=True,
                                tile_position=(32 * c, 0),
                                skip_group_check=True,
                            )
                        for c in range(HC):
                            nc.vector.tensor_copy(
                                z2s_t[s][:, c, :], z2ps[c][:]
                            )

    nc.compile()
    return nc


def _hi_lo(a32):
    hi = a32.astype(BF16)
    lo = (a32 - hi.astype(np.float32)).astype(BF16)
    return hi, lo


def kernel(
    t,
    state,
    action_emb,
    W1,
    b1,
    W2,
    b2,
    W3,
    b3,
    log_damping,
    Gw,
    Gb,
):
    global last_results
    import os
    from concourse.bass_utils import run_bass_kernel_spmd

    state = np.asarray(state, dtype=np.float32)
    action_emb = np.asarray(action_emb, dtype=np.float32)
    W1 = np.asarray(W1, dtype=np.float32)
    b1 = np.asarray(b1, dtype=np.float32)
    W2 = np.asarray(W2, dtype=np.float32)
    b2 = np.asarray(b2, dtype=np.float32)
    W3 = np.asarray(W3, dtype=np.float32)
    b3 = np.asarray(b3, dtype=np.float32)  # unused: constant shift, no grad
    damping = float(np.exp(np.float32(log_damping)))
    Gw = np.asarray(Gw, dtype=np.float32)
    Gb = np.asarray(Gb, dtype=np.float32)

    # ---- host-side weight prep (tiny) ----
    w3col = W3[:, 0]
    w1hi, w1lo = _hi_lo(W1)  # [2, H] each
    w1a = np.concatenate(
        [w1hi, w1hi, w1lo, b1[None, :].astype(BF16)], axis=0
    )  # [7, H] bf16
    # row-tiled layout: rows 32j+r = w1a[r, 128j:128j+128]
    w1rt = np.zeros((128, 128), dtype=BF16)
    for j in range(4):
        w1rt[32 * j : 32 * j + 7, :] = w1a[:, 128 * j : 128 * (j + 1)]

    # z2 weights: fp8(-s_w W2), [p, kc, i] = q[kc*128+p, i]
    w2q = (
        (-S_W * W2).astype(F8).reshape(HC, 128, H).transpose(1, 0, 2).copy()
    )
    # u weights: fp8(s_u/2 * Wb), Wb[j, i] = W2[i, j] w3[j]
    w2wt = W2.T * w3col[:, None]  # [H(j), H(i)]
    wuq = (
        (0.5 * S_U * w2wt).astype(F8).reshape(HC, 128, H).transpose(1, 0, 2).copy()
    )
    # out weights: -(M@W1)/s_u, [p, kc, c] layout
    M = np.array([[0.0, 1.0], [-1.0, -damping]], dtype=np.float32)
    w1f = -(M @ W1) / np.float32(S_U)  # [2, H]
    w1ftr = w1f.T.astype(BF16).reshape(HC, 128, S).transpose(1, 0, 2).copy()

    gmat = np.zeros((17, S), dtype=np.float32)
    gmat[0:8, 1] = Gw[:, 0]
    gmat[8:16, 1] = Gw[:, 0]
    gmat[16, 1] = Gb[0]
    gmat = gmat.astype(BF16)

    # const-fold weights: b2 (into z2 psum) and u0 (into u psum), hi/lo
    b2s = (S_W * b2).astype(np.float32).reshape(HC, 128)
    u0s = (0.5 * S_U * w2wt.sum(axis=0)).astype(np.float32).reshape(HC, 128)
    cwb = np.zeros((128, 128), dtype=BF16)
    cwu = np.zeros((128, 128), dtype=BF16)
    for c in range(HC):
        bhi, blo = _hi_lo(b2s[c])
        uhi, ulo = _hi_lo(u0s[c])
        cwb[32 * c + 0, :] = bhi
        cwb[32 * c + 1, :] = blo
        cwu[32 * c + 0, :] = uhi
        cwu[32 * c + 1, :] = ulo

    # ---- per-core input shards ----
    sT = state.T  # [2, B]
    shi, slo = _hi_lo(sT)
    ones_row = np.ones((1, B), dtype=BF16)
    xT = np.concatenate([shi, slo, shi, ones_row], axis=0)  # [7, B]

    aT32 = action_emb.T  # [8, B]
    ahi, alo = _hi_lo(aT32)
    aT = np.concatenate([ahi, alo, ones_row], axis=0)  # [17, B]

    if "nc" not in _cached:
        _cached["nc"] = _build_nc()
    nc = _cached["nc"]

    in_maps = []
    for c in range(NCORES):
        csl = slice(c * BC, (c + 1) * BC)
        in_maps.append(
            {
                "xT": np.ascontiguousarray(xT[:, csl]),
                "aT": np.ascontiguousarray(aT[:, csl]),
                "w1rt": w1rt,
                "w2q": w2q,
                "wuq": wuq,
                "w1f": w1ftr,
                "gm": gmat,
                "cwb": cwb,
                "cwu": cwu,
            }
        )

    trace = bool(os.environ.get("PH_TRACE"))
    res = run_bass_kernel_spmd(
        nc, in_maps, core_ids=list(range(NCORES)), trace=trace
    )
    last_results = res

    out = np.empty((B, S), dtype=np.float32)
    for c in range(NCORES):
        out[c * BC : (c + 1) * BC, :] = res.results[c]["outT"].T
    return out
